# revision 50
# baseline (speedup 1.0000x reference)
"""Causal self-attention Trainium2 kernel (8-core head-parallel).

Full inputs in, full output out. Sharding strategy:
  - 16 heads / 8 cores -> 2 heads per core, both batch elems (4 (b,h) attention
    problems per core).
  - QKV projection column-parallel: each core gets w_attn[:, cols-of-its-heads]
    as a [1024, 384] slice (q 128 | k 128 | v 128), q pre-scaled by 1/sqrt(D).
  - c_proj row-parallel: each core gets w_proj[128c:128c+128, :] and produces a
    partial [B, T, C] output; host sums the 8 partials (the all-reduce of the
    row-parallel projection) and adds the bias.

Schedule (single phase-A + phase-B pipeline, no fat warm-up matmuls):
  Phase A: x^T streams in token-major [128, 512/1024] pieces; QKV runs in
  512-token groups with q/k/v matmuls interleaved per contraction tile so the
  PE is fed as DMA lands.  v^T tiles are PE-transposed into a V slab
  [128 s, 32, 130] (h0 d | ones | h1 d) so each AV stationary is a contiguous
  65-column slice whose ones column yields the softmax denominator Z for free.
  Phase B: attention in 512-wide q windows; both heads share one [128,2,512]
  S psum tile -> one exp per s-tile; causal masking via a triangular multiply
  on GPSIMD for diagonal tiles only.  Normalization uses DVE reciprocal on the
  Z row + a PE broadcast matmul.  Output projection takes normalized y^T as
  stationary and w_proj rows as moving, producing out[b, tok, :] tiles
  directly in [B, T, C] orientation.
"""

import math

import numpy as np
import ml_dtypes

import concourse.bass as bass
from concourse import bacc
import concourse.mybir as mybir
from concourse.tile import TileContext
from concourse.bass_utils import run_bass_kernel_spmd

BF16 = mybir.dt.bfloat16
F32 = mybir.dt.float32
NPBF16 = ml_dtypes.bfloat16

P = 128
B, T, C = 2, 2048, 1024
H, D = 16, 64
NCORES = 8
HPC = H // NCORES          # heads per core
TOK = B * T                # 4096 flattened tokens (b-major)
NCT = C // P               # 8 contraction tiles for the projections
QW = 512                   # q window width for attention
NG = TOK // QW             # 8 token groups of 512 for the QKV projection
NST = TOK // P             # 32 token tiles of 128
EXP_BIAS = -4.0            # exp(s - 4): cancels in normalization, guards tail


def build_nc(with_bias: bool) -> bacc.Bacc:
    nc = bacc.Bacc(None, target_bir_lowering=False)

    xt = nc.dram_tensor("xt", [C, TOK], BF16, kind="ExternalInput")
    wqkv = nc.dram_tensor("wqkv", [C, 3 * P], BF16, kind="ExternalInput")
    wp = nc.dram_tensor("wp", [P, C], BF16, kind="ExternalInput")
    tri = nc.dram_tensor("tri", [P, P], BF16, kind="ExternalInput")
    ident = nc.dram_tensor("ident", [P, P], BF16, kind="ExternalInput")
    ones64 = nc.dram_tensor("ones64", [1, 64], BF16, kind="ExternalInput")
    if with_bias:
        bqkv = nc.dram_tensor("bqkv", [1, 3 * P], BF16, kind="ExternalInput")
        ones512 = nc.dram_tensor("ones512", [1, 512], BF16, kind="ExternalInput")
    out = nc.dram_tensor("out", [B, T, C], BF16, kind="ExternalOutput")

    EXP = mybir.ActivationFunctionType.Exp

    with TileContext(nc) as tc:
        with (
            tc.tile_pool(name="consts", bufs=1) as consts,
            tc.tile_pool(name="px", bufs=1) as px,
            tc.tile_pool(name="pqkv", bufs=1) as pqkv,
            tc.tile_pool(name="py", bufs=1) as py,
            tc.tile_pool(name="pwork", bufs=2) as pwork,
        ):
            # ---- constant loads on the ACT queue (xt owns the sync queue) ----
            # one fused DMA: dram row ct*P+p -> sbuf partition p, free (ct, :)
            wqkv_sb = consts.tile([P, NCT, 3 * P], BF16)
            nc.scalar.dma_start(
                wqkv_sb, wqkv[:, :].rearrange("(ct p) f -> p ct f", ct=NCT))
            wp_sb = consts.tile([P, C], BF16)
            nc.scalar.dma_start(wp_sb, wp[:, :])
            tri_sb = consts.tile([P, P], BF16)
            nc.scalar.dma_start(tri_sb, tri[:, :])
            ident_sb = consts.tile([P, P], BF16)
            nc.scalar.dma_start(ident_sb, ident[:, :])
            ones64_sb = consts.tile([1, 64], BF16)
            nc.scalar.dma_start(ones64_sb, ones64[:, :])
            expb = consts.tile([P, 1], F32)
            nc.vector.memset(expb, EXP_BIAS)
            warm_sb = consts.tile([P, P], BF16)
            nc.vector.memset(warm_sb, 1.0)
            if with_bias:
                bqkv_sb = consts.tile([1, 3 * P], BF16)
                nc.scalar.dma_start(bqkv_sb, bqkv[:, :])
                ones512_sb = consts.tile([1, 512], BF16)
                nc.scalar.dma_start(ones512_sb, ones512[:, :])

            # ---- x^T streamed token-major over two issue queues (sync takes
            # even ct, ACT odd ct) so descriptor-issue rate never starves the
            # QKV groups and transfers spread across DMA engines ----
            xt_sb = px.tile([P, NCT, TOK], BF16)
            for lo, hi in ((0, 512), (512, 1024), (1024, 2048),
                           (2048, 4096)):
                for ct in range(NCT):
                    eng = nc.sync if ct % 2 == 0 else nc.scalar
                    eng.dma_start(xt_sb[:, ct, lo:hi],
                                  xt[ct * P:(ct + 1) * P, lo:hi])

            # qkv^T slab: ft 0 q^T, 1 k^T, 2 v^T
            qkT = pqkv.tile([P, 3, TOK], BF16)
            vT = qkT[:, 2, :]
            # V slab: [s, tile, 131] = h0 d 0:64 | ones | h1 d 65:129 | ones
            V = pqkv.tile([P, NST, 131], BF16)
            nc.vector.memset(V[:, :, 64:65], 1.0)
            nc.vector.memset(V[:, :, 129:130], 1.0)
            yT = py.tile([P, B, T], BF16)

            # ---- phase A: QKV projection, token-major groups ----
            with (
                tc.tile_pool(name="ps_qkv", bufs=1, space="PSUM") as ps_qkv,
                tc.tile_pool(name="ps_t", bufs=1, space="PSUM") as ps_t,
            ):
                wn = [0]

                def warm(n):
                    # PE warm-up that draws little power: transpose matmuls of
                    # an sbuf constant (pass-through, no MAC toggling). The
                    # tiny DVE read defeats walrus dead-code elimination.
                    for _ in range(n):
                        wn[0] += 1
                        pt = ps_t.tile([P, P], BF16, tag="vt", bufs=2,
                                       name=f"warmps_{wn[0]}")
                        nc.tensor.transpose(pt, warm_sb, warm_sb)
                        wr = pwork.tile([1, 4], BF16, tag="wr", bufs=2,
                                        name=f"wr_{wn[0]}")
                        nc.vector.tensor_copy(wr, pt[0:1, 0:4])

                warm(12)

                for g in range(NG):
                    gl, gh = g * QW, (g + 1) * QW
                    ps3 = ps_qkv.tile([P, 3, QW], F32, tag="qkv", bufs=2,
                                      name=f"qkvps_{g}")
                    for ct in range(NCT):
                        for ft in range(3):
                            nc.tensor.matmul(
                                ps3[:, ft, :],
                                wqkv_sb[:, ct, ft * P:(ft + 1) * P],
                                xt_sb[:, ct, gl:gh],
                                start=(ct == 0),
                                stop=(ct == NCT - 1 and not with_bias),
                            )
                        if g <= 1:
                            warm(1)
                    if with_bias:
                        for ft in range(3):
                            nc.tensor.matmul(
                                ps3[:, ft, :],
                                bqkv_sb[0:1, ft * P:(ft + 1) * P],
                                ones512_sb[0:1, :],
                                start=False, stop=True,
                            )
                    nc.scalar.copy(qkT[:, :, gl:gh], ps3)
                    # v^T -> V slab tiles for this group (both heads per tile)
                    for st4 in range(QW // P):
                        si = g * (QW // P) + st4
                        pt = ps_t.tile([P, P], BF16, tag="vt", bufs=2,
                                       name=f"vtps_{si}")
                        nc.tensor.transpose(
                            pt, vT[:, si * P:(si + 1) * P], ident_sb)
                        nc.vector.tensor_copy(V[:, si, 0:64], pt[:, 0:64])
                        nc.vector.tensor_copy(V[:, si, 65:129], pt[:, 64:128])

            # ---- phase B: attention + normalize + projection ----
            with tc.tile_pool(name="ps_att", bufs=1, space="PSUM") as ps_att:
                pending = []           # deferred norm+proj stages of prev window
                otn = [0]

                def emit_proj(b, w, tts=None):
                    qbase = w * QW
                    for tt in (range(QW // P) if tts is None else tts):
                        t0 = qbase + tt * P
                        pp = ps_att.tile([P, QW], F32, tag="proj", bufs=2,
                                         name=f"pp_{b}_{w}_{tt}")
                        ot = pwork.tile([P, C], BF16, tag="ot", bufs=4,
                                        name=f"ot_{b}_{w}_{tt}")
                        for c in range(2):
                            nc.tensor.matmul(
                                pp[:, 0:512],
                                yT[:, b, t0:t0 + P],
                                wp_sb[:, c * 512:(c + 1) * 512],
                                start=True, stop=True,
                            )
                            if otn[0] % 3 != 2:
                                nc.vector.tensor_copy(
                                    ot[:, c * 512:(c + 1) * 512], pp[:, 0:512])
                            else:
                                nc.scalar.copy(
                                    ot[:, c * 512:(c + 1) * 512], pp[:, 0:512])
                            otn[0] += 1
                        nc.sync.dma_start(out[b, t0:t0 + P, :], ot)

                def make_norm(b, w, ys):
                    """Stage closures: [recip+rb, ynum+mul+proj]."""
                    qbase = w * QW
                    rs, rbs = [], []

                    def stage1():
                        for h in range(HPC):
                            # approx-recip ucode breaks on non-zero partition
                            # base: stage Z at partition 0 first
                            zr = pwork.tile([1, QW], F32, tag="zr", bufs=4,
                                            name=f"zr_{b}_{w}_{h}")
                            nc.vector.tensor_copy(zr, ys[h][64:65, :])
                            rf = pwork.tile([1, QW], F32, tag="rf", bufs=4,
                                            name=f"rf_{b}_{w}_{h}")
                            nc.vector.reciprocal_approx_fast(rf, zr)
                            r = pwork.tile([1, QW], BF16, tag="r", bufs=4,
                                           name=f"r_{b}_{w}_{h}")
                            nc.scalar.copy(r, rf)
                            rb = ps_att.tile([P, QW], F32, tag="proj",
                                             bufs=2, name=f"rb_{b}_{w}_{h}")
                            nc.tensor.matmul(rb[0:64, :], ones64_sb, r,
                                             start=True, stop=True)
                            rs.append(r)
                            rbs.append(rb)

                    def stage2(c0=0, c1=QW, tts=None):
                        for h in range(HPC):
                            ynum = pwork.tile([64, c1 - c0], BF16, tag="ynum",
                                              bufs=4,
                                              name=f"yn_{b}_{w}_{h}_{c0}")
                            nc.vector.tensor_copy(ynum, ys[h][0:64, c0:c1])
                            nc.vector.tensor_mul(
                                yT[h * 64:(h + 1) * 64, b,
                                   qbase + c0:qbase + c1],
                                ynum, rbs[h][0:64, c0:c1])
                        emit_proj(b, w, tts)

                    if (b, w) == (B - 1, T // QW - 1):
                        # final window: drain in halves so the first output
                        # projections and DMAs overlap the second normalize
                        return [stage1,
                                lambda: stage2(0, QW // 2, (0, 1)),
                                lambda: stage2(QW // 2, QW, (2, 3))]
                    return [stage1, stage2]

                for b in range(B):
                    for w in range(T // QW):
                        qbase = w * QW
                        n_st = (qbase + QW) // P
                        # lazy y allocation: first AV (emitted after the
                        # previous window's norm stages) grabs the slots, so
                        # slot reuse correctly waits on those stages' reads
                        ys = {}

                        def get_ys(h, b=b, w=w, ys=ys):
                            if h not in ys:
                                ys[h] = ps_att.tile(
                                    [P, QW], F32, tag="y", bufs=2,
                                    name=f"yps_{b}_{w}_{h}")
                            return ys[h]

                        av_fifo = []
                        for st in range(n_st):
                            if pending and st in (0, 2):
                                pending.pop(0)()
                            s0 = st * P
                            qa = max(qbase, s0)
                            wdt = qbase + QW - qa
                            ps = ps_att.tile([P, 2, QW], F32, tag="S", bufs=2,
                                             name=f"sps_{b}_{w}_{st}")
                            for h in range(HPC):
                                nc.tensor.matmul(
                                    ps[:, h, 0:wdt],
                                    qkT[h * 64:(h + 1) * 64, 1,
                                        b * T + s0: b * T + s0 + P],
                                    qkT[h * 64:(h + 1) * 64, 0,
                                        b * T + qa: b * T + qa + wdt],
                                    start=True, stop=True,
                                )
                            es = pwork.tile([P, 2, QW], BF16, tag="es", bufs=6,
                                            name=f"es_{b}_{w}_{st}")
                            nc.scalar.activation(
                                es[:, :, 0:wdt], ps[:, :, 0:wdt], EXP,
                                bias=expb)
                            if s0 >= qbase:      # diagonal tile: causal mask
                                # DVE, all-bf16 SBUF operands -> 4x fast path
                                for h in range(HPC):
                                    nc.vector.tensor_mul(
                                        es[:, h, 0:P], es[:, h, 0:P], tri_sb)

                            def av(st=st, es=es, off=qa - qbase, wdt=wdt,
                                   last=(st == n_st - 1), first=(st == 0),
                                   vi=b * (T // P) + st, get_ys=get_ys):
                                for h in range(HPC):
                                    nc.tensor.matmul(
                                        get_ys(h)[0:65, off:off + wdt],
                                        V[:, vi, h * 65: h * 65 + 65],
                                        es[:, h, 0:wdt],
                                        start=first, stop=last,
                                    )
                            av_fifo.append(av)
                            if len(av_fifo) > 3:
                                av_fifo.pop(0)()
                        for f in av_fifo:
                            f()
                        pending = make_norm(b, w, [ys[h] for h in range(HPC)])
                for f in pending:
                    f()
    nc.compile()
    return nc


_CACHE = {}


def _get_nc(with_bias: bool) -> bacc.Bacc:
    if with_bias not in _CACHE:
        _CACHE[with_bias] = build_nc(with_bias)
    return _CACHE[with_bias]


def _prep_inputs(x, w_attn, b_attn, w_proj):
    """Host-side shard + layout prep. Returns per-core in_maps."""
    xf = np.ascontiguousarray(
        np.asarray(x, dtype=np.float32).reshape(TOK, C).T
    ).astype(NPBF16)                                   # x^T [C, TOK]
    w = np.asarray(w_attn, dtype=np.float32)
    ba = np.asarray(b_attn, dtype=np.float32)
    wpj = np.asarray(w_proj, dtype=np.float32)
    scale = 1.0 / math.sqrt(D)
    with_bias = bool(np.any(ba))

    tri_np = np.triu(np.ones((P, P), dtype=np.float32)).astype(NPBF16)
    id_np = np.eye(P, dtype=np.float32).astype(NPBF16)
    ones64_np = np.ones((1, 64), dtype=np.float32).astype(NPBF16)
    ones512_np = np.ones((1, 512), dtype=np.float32).astype(NPBF16)

    in_maps = []
    for c in range(NCORES):
        lo, hi = c * HPC * D, (c + 1) * HPC * D        # 128-wide head slice
        wq = w[:, lo:hi] * scale
        wk = w[:, C + lo:C + hi]
        wv = w[:, 2 * C + lo:2 * C + hi]
        wqkv_c = np.concatenate([wq, wk, wv], axis=1).astype(NPBF16)
        wp_c = np.ascontiguousarray(wpj[lo:hi, :]).astype(NPBF16)
        m = {
            "xt": xf,
            "wqkv": wqkv_c,
            "wp": wp_c,
            "tri": tri_np,
            "ident": id_np,
            "ones64": ones64_np,
        }
        if with_bias:
            bq = ba[lo:hi] * scale
            bk = ba[C + lo:C + hi]
            bv = ba[2 * C + lo:2 * C + hi]
            m["bqkv"] = np.concatenate([bq, bk, bv])[None, :].astype(NPBF16)
            m["ones512"] = ones512_np
        in_maps.append(m)
    return in_maps, with_bias


def _combine(results, b_proj):
    acc = np.zeros((B, T, C), dtype=np.float32)
    for r in results:
        acc += np.asarray(r["out"], dtype=np.float32)
    acc += np.asarray(b_proj, dtype=np.float32)[None, None, :]
    return np.ascontiguousarray(acc)


def run(x, w_attn, b_attn, w_proj, b_proj, trace=False, trace_cores=None):
    in_maps, with_bias = _prep_inputs(x, w_attn, b_attn, w_proj)
    nc = _get_nc(with_bias)
    res = run_bass_kernel_spmd(
        nc, in_maps, core_ids=list(range(NCORES)),
        trace=trace, trace_cores=trace_cores,
    )
    return _combine(res.results, b_proj), res


def kernel(x, w_attn, b_attn, w_proj, b_proj):
    out, _ = run(x, w_attn, b_attn, w_proj, b_proj, trace=False)
    return out


# revision 51
# speedup vs baseline: 1.0146x; 1.0146x over previous
"""Causal self-attention Trainium2 kernel (8-core head-parallel).

Full inputs in, full output out. Sharding strategy:
  - 16 heads / 8 cores -> 2 heads per core, both batch elems (4 (b,h) attention
    problems per core).
  - QKV projection column-parallel: each core gets w_attn[:, cols-of-its-heads]
    as a [1024, 384] slice (q 128 | k 128 | v 128), q pre-scaled by 1/sqrt(D).
  - c_proj row-parallel: each core gets w_proj[128c:128c+128, :] and produces a
    partial [B, T, C] output; host sums the 8 partials (the all-reduce of the
    row-parallel projection) and adds the bias.

Schedule (single phase-A + phase-B pipeline, no fat warm-up matmuls):
  Phase A: x^T streams in token-major [128, 512/1024] pieces; QKV runs in
  512-token groups with q/k/v matmuls interleaved per contraction tile so the
  PE is fed as DMA lands.  v^T tiles are PE-transposed into a V slab
  [128 s, 32, 130] (h0 d | ones | h1 d) so each AV stationary is a contiguous
  65-column slice whose ones column yields the softmax denominator Z for free.
  Phase B: attention in 512-wide q windows; both heads share one [128,2,512]
  S psum tile -> one exp per s-tile; causal masking via a triangular multiply
  on GPSIMD for diagonal tiles only.  Normalization uses DVE reciprocal on the
  Z row + a PE broadcast matmul.  Output projection takes normalized y^T as
  stationary and w_proj rows as moving, producing out[b, tok, :] tiles
  directly in [B, T, C] orientation.
"""

import math

import numpy as np
import ml_dtypes

import concourse.bass as bass
from concourse import bacc
import concourse.mybir as mybir
from concourse.tile import TileContext
from concourse.bass_utils import run_bass_kernel_spmd

BF16 = mybir.dt.bfloat16
F32 = mybir.dt.float32
NPBF16 = ml_dtypes.bfloat16

P = 128
B, T, C = 2, 2048, 1024
H, D = 16, 64
NCORES = 8
HPC = H // NCORES          # heads per core
TOK = B * T                # 4096 flattened tokens (b-major)
NCT = C // P               # 8 contraction tiles for the projections
QW = 512                   # q window width for attention
NG = TOK // QW             # 8 token groups of 512 for the QKV projection
NST = TOK // P             # 32 token tiles of 128
EXP_BIAS = -4.0            # exp(s - 4): cancels in normalization, guards tail


def build_nc(with_bias: bool) -> bacc.Bacc:
    nc = bacc.Bacc(None, target_bir_lowering=False)

    xt = nc.dram_tensor("xt", [C, TOK], BF16, kind="ExternalInput")
    wqkv = nc.dram_tensor("wqkv", [C, 3 * P], BF16, kind="ExternalInput")
    wp = nc.dram_tensor("wp", [P, C], BF16, kind="ExternalInput")
    tri = nc.dram_tensor("tri", [P, P], BF16, kind="ExternalInput")
    ident = nc.dram_tensor("ident", [P, P], BF16, kind="ExternalInput")
    ones64 = nc.dram_tensor("ones64", [1, 64], BF16, kind="ExternalInput")
    if with_bias:
        bqkv = nc.dram_tensor("bqkv", [1, 3 * P], BF16, kind="ExternalInput")
        ones512 = nc.dram_tensor("ones512", [1, 512], BF16, kind="ExternalInput")
    out = nc.dram_tensor("out", [B, T, C], BF16, kind="ExternalOutput")

    EXP = mybir.ActivationFunctionType.Exp

    with TileContext(nc) as tc:
        with (
            tc.tile_pool(name="consts", bufs=1) as consts,
            tc.tile_pool(name="px", bufs=1) as px,
            tc.tile_pool(name="pqkv", bufs=1) as pqkv,
            tc.tile_pool(name="py", bufs=1) as py,
            tc.tile_pool(name="pwork", bufs=2) as pwork,
        ):
            # ---- constant loads on the ACT queue (xt owns the sync queue) ----
            # one fused DMA: dram row ct*P+p -> sbuf partition p, free (ct, :)
            wqkv_sb = consts.tile([P, NCT, 3 * P], BF16)
            nc.scalar.dma_start(
                wqkv_sb, wqkv[:, :].rearrange("(ct p) f -> p ct f", ct=NCT))
            wp_sb = consts.tile([P, C], BF16)
            nc.scalar.dma_start(wp_sb, wp[:, :])
            tri_sb = consts.tile([P, P], BF16)
            nc.scalar.dma_start(tri_sb, tri[:, :])
            ident_sb = consts.tile([P, P], BF16)
            nc.scalar.dma_start(ident_sb, ident[:, :])
            ones64_sb = consts.tile([1, 64], BF16)
            nc.scalar.dma_start(ones64_sb, ones64[:, :])
            expb = consts.tile([P, 1], F32)
            nc.vector.memset(expb, EXP_BIAS)
            warm_sb = consts.tile([P, P], BF16)
            nc.vector.memset(warm_sb, 1.0)
            if with_bias:
                bqkv_sb = consts.tile([1, 3 * P], BF16)
                nc.scalar.dma_start(bqkv_sb, bqkv[:, :])
                ones512_sb = consts.tile([1, 512], BF16)
                nc.scalar.dma_start(ones512_sb, ones512[:, :])

            # ---- x^T streamed token-major over two issue queues (sync takes
            # even ct, ACT odd ct) so descriptor-issue rate never starves the
            # QKV groups and transfers spread across DMA engines ----
            xt_sb = px.tile([P, NCT, TOK], BF16)
            for lo, hi in ((0, 512), (512, 1024), (1024, 2048),
                           (2048, 4096)):
                for ct in range(NCT):
                    eng = nc.sync if ct % 2 == 0 else nc.scalar
                    eng.dma_start(xt_sb[:, ct, lo:hi],
                                  xt[ct * P:(ct + 1) * P, lo:hi])

            # qkv^T slab: ft 0 q^T, 1 k^T, 2 v^T
            qkT = pqkv.tile([P, 3, TOK], BF16)
            vT = qkT[:, 2, :]
            # V slab: [s, tile, 131] = h0 d 0:64 | ones | h1 d 65:129 | ones
            V = pqkv.tile([P, NST, 131], BF16)
            nc.vector.memset(V[:, :, 64:65], 1.0)
            nc.vector.memset(V[:, :, 129:130], 1.0)
            yT = py.tile([P, B, T], BF16)

            # ---- phase A: QKV projection, token-major groups ----
            with (
                tc.tile_pool(name="ps_qkv", bufs=1, space="PSUM") as ps_qkv,
                tc.tile_pool(name="ps_t", bufs=1, space="PSUM") as ps_t,
            ):
                wn = [0]

                def warm(n):
                    # PE warm-up that draws little power: transpose matmuls of
                    # an sbuf constant (pass-through, no MAC toggling). The
                    # tiny DVE read defeats walrus dead-code elimination.
                    for _ in range(n):
                        wn[0] += 1
                        pt = ps_t.tile([P, P], BF16, tag="vt", bufs=2,
                                       name=f"warmps_{wn[0]}")
                        nc.tensor.transpose(pt, warm_sb, warm_sb)
                        wr = pwork.tile([1, 4], BF16, tag="wr", bufs=2,
                                        name=f"wr_{wn[0]}")
                        nc.vector.tensor_copy(wr, pt[0:1, 0:4])

                warm(12)

                for g in range(NG):
                    gl, gh = g * QW, (g + 1) * QW
                    ps3 = ps_qkv.tile([P, 3, QW], F32, tag="qkv", bufs=2,
                                      name=f"qkvps_{g}")
                    for ct in range(NCT):
                        for ft in range(3):
                            nc.tensor.matmul(
                                ps3[:, ft, :],
                                wqkv_sb[:, ct, ft * P:(ft + 1) * P],
                                xt_sb[:, ct, gl:gh],
                                start=(ct == 0),
                                stop=(ct == NCT - 1 and not with_bias),
                            )
                        if g <= 1:
                            warm(1)
                    if with_bias:
                        for ft in range(3):
                            nc.tensor.matmul(
                                ps3[:, ft, :],
                                bqkv_sb[0:1, ft * P:(ft + 1) * P],
                                ones512_sb[0:1, :],
                                start=False, stop=True,
                            )
                    nc.scalar.copy(qkT[:, :, gl:gh], ps3)
                    # v^T -> V slab tiles for this group (both heads per tile)
                    for st4 in range(QW // P):
                        si = g * (QW // P) + st4
                        pt = ps_t.tile([P, P], BF16, tag="vt", bufs=2,
                                       name=f"vtps_{si}")
                        nc.tensor.transpose(
                            pt, vT[:, si * P:(si + 1) * P], ident_sb)
                        nc.vector.tensor_copy(V[:, si, 0:64], pt[:, 0:64])
                        nc.vector.tensor_copy(V[:, si, 65:129], pt[:, 64:128])

            # ---- phase B: attention + normalize + projection ----
            with tc.tile_pool(name="ps_att", bufs=1, space="PSUM") as ps_att:
                pending = []           # deferred norm+proj stages of prev window
                otn = [0]

                def emit_proj(b, w, tts=None):
                    qbase = w * QW
                    for tt in (range(QW // P) if tts is None else tts):
                        t0 = qbase + tt * P
                        pp = ps_att.tile([P, QW], F32, tag="proj", bufs=2,
                                         name=f"pp_{b}_{w}_{tt}")
                        ot = pwork.tile([P, C], BF16, tag="ot", bufs=4,
                                        name=f"ot_{b}_{w}_{tt}")
                        for c in range(2):
                            nc.tensor.matmul(
                                pp[:, 0:512],
                                yT[:, b, t0:t0 + P],
                                wp_sb[:, c * 512:(c + 1) * 512],
                                start=True, stop=True,
                            )
                            if otn[0] % 3 != 2:
                                nc.vector.tensor_copy(
                                    ot[:, c * 512:(c + 1) * 512], pp[:, 0:512])
                            else:
                                nc.scalar.copy(
                                    ot[:, c * 512:(c + 1) * 512], pp[:, 0:512])
                            otn[0] += 1
                        nc.sync.dma_start(out[b, t0:t0 + P, :], ot)

                def make_norm(b, w, ys):
                    """Stage closures: [recip+rb, ynum+mul+proj]."""
                    qbase = w * QW
                    rs, rbs = [], []

                    def stage1():
                        for h in range(HPC):
                            # approx-recip ucode breaks on non-zero partition
                            # base: stage Z at partition 0 first
                            zr = pwork.tile([1, QW], F32, tag="zr", bufs=4,
                                            name=f"zr_{b}_{w}_{h}")
                            nc.vector.tensor_copy(zr, ys[h][64:65, :])
                            rf = pwork.tile([1, QW], F32, tag="rf", bufs=4,
                                            name=f"rf_{b}_{w}_{h}")
                            nc.vector.reciprocal_approx_fast(rf, zr)
                            r = pwork.tile([1, QW], BF16, tag="r", bufs=4,
                                           name=f"r_{b}_{w}_{h}")
                            nc.scalar.copy(r, rf)
                            rb = ps_att.tile([P, QW], F32, tag="proj",
                                             bufs=2, name=f"rb_{b}_{w}_{h}")
                            nc.tensor.matmul(rb[0:64, :], ones64_sb, r,
                                             start=True, stop=True)
                            rs.append(r)
                            rbs.append(rb)

                    def stage2(c0=0, c1=QW, tts=None):
                        for h in range(HPC):
                            ynum = pwork.tile([64, c1 - c0], BF16, tag="ynum",
                                              bufs=4,
                                              name=f"yn_{b}_{w}_{h}_{c0}")
                            nc.vector.tensor_copy(ynum, ys[h][0:64, c0:c1])
                            nc.vector.tensor_mul(
                                yT[h * 64:(h + 1) * 64, b,
                                   qbase + c0:qbase + c1],
                                ynum, rbs[h][0:64, c0:c1])
                        emit_proj(b, w, tts)

                    if (b, w) == (B - 1, T // QW - 1):
                        # final window: drain in halves so the first output
                        # projections and DMAs overlap the second normalize
                        return [stage1,
                                lambda: stage2(0, QW // 2, (0, 1)),
                                lambda: stage2(QW // 2, QW, (2, 3))]
                    return [stage1, stage2]

                for b in range(B):
                    for w in range(T // QW):
                        qbase = w * QW
                        n_st = (qbase + QW) // P
                        # lazy y allocation: first AV (emitted after the
                        # previous window's norm stages) grabs the slots, so
                        # slot reuse correctly waits on those stages' reads
                        ys = {}

                        def get_ys(h, b=b, w=w, ys=ys):
                            if h not in ys:
                                ys[h] = ps_att.tile(
                                    [P, QW], F32, tag="y", bufs=2,
                                    name=f"yps_{b}_{w}_{h}")
                            return ys[h]

                        av_fifo = []
                        for st in range(n_st):
                            if pending and st in (0, 2):
                                pending.pop(0)()
                            s0 = st * P
                            qa = max(qbase, s0)
                            wdt = qbase + QW - qa
                            ps = ps_att.tile([P, 2, QW], F32, tag="S", bufs=2,
                                             name=f"sps_{b}_{w}_{st}")
                            for h in range(HPC):
                                nc.tensor.matmul(
                                    ps[:, h, 0:wdt],
                                    qkT[h * 64:(h + 1) * 64, 1,
                                        b * T + s0: b * T + s0 + P],
                                    qkT[h * 64:(h + 1) * 64, 0,
                                        b * T + qa: b * T + qa + wdt],
                                    start=True, stop=True,
                                )
                            es = pwork.tile([P, 2, QW], BF16, tag="es", bufs=8,
                                            name=f"es_{b}_{w}_{st}")
                            nc.scalar.activation(
                                es[:, :, 0:wdt], ps[:, :, 0:wdt], EXP,
                                bias=expb)
                            if s0 >= qbase:      # diagonal tile: causal mask
                                # DVE, all-bf16 SBUF operands -> 4x fast path
                                for h in range(HPC):
                                    nc.vector.tensor_mul(
                                        es[:, h, 0:P], es[:, h, 0:P], tri_sb)

                            def av(st=st, es=es, off=qa - qbase, wdt=wdt,
                                   last=(st == n_st - 1), first=(st == 0),
                                   vi=b * (T // P) + st, get_ys=get_ys):
                                for h in range(HPC):
                                    nc.tensor.matmul(
                                        get_ys(h)[0:65, off:off + wdt],
                                        V[:, vi, h * 65: h * 65 + 65],
                                        es[:, h, 0:wdt],
                                        start=first, stop=last,
                                    )
                            av_fifo.append(av)
                            if len(av_fifo) > 3:
                                av_fifo.pop(0)()
                        for f in av_fifo:
                            f()
                        pending = make_norm(b, w, [ys[h] for h in range(HPC)])
                for f in pending:
                    f()
    nc.compile()
    return nc


_CACHE = {}


def _get_nc(with_bias: bool) -> bacc.Bacc:
    if with_bias not in _CACHE:
        _CACHE[with_bias] = build_nc(with_bias)
    return _CACHE[with_bias]


def _prep_inputs(x, w_attn, b_attn, w_proj):
    """Host-side shard + layout prep. Returns per-core in_maps."""
    xf = np.ascontiguousarray(
        np.asarray(x, dtype=np.float32).reshape(TOK, C).T
    ).astype(NPBF16)                                   # x^T [C, TOK]
    w = np.asarray(w_attn, dtype=np.float32)
    ba = np.asarray(b_attn, dtype=np.float32)
    wpj = np.asarray(w_proj, dtype=np.float32)
    scale = 1.0 / math.sqrt(D)
    with_bias = bool(np.any(ba))

    tri_np = np.triu(np.ones((P, P), dtype=np.float32)).astype(NPBF16)
    id_np = np.eye(P, dtype=np.float32).astype(NPBF16)
    ones64_np = np.ones((1, 64), dtype=np.float32).astype(NPBF16)
    ones512_np = np.ones((1, 512), dtype=np.float32).astype(NPBF16)

    in_maps = []
    for c in range(NCORES):
        lo, hi = c * HPC * D, (c + 1) * HPC * D        # 128-wide head slice
        wq = w[:, lo:hi] * scale
        wk = w[:, C + lo:C + hi]
        wv = w[:, 2 * C + lo:2 * C + hi]
        wqkv_c = np.concatenate([wq, wk, wv], axis=1).astype(NPBF16)
        wp_c = np.ascontiguousarray(wpj[lo:hi, :]).astype(NPBF16)
        m = {
            "xt": xf,
            "wqkv": wqkv_c,
            "wp": wp_c,
            "tri": tri_np,
            "ident": id_np,
            "ones64": ones64_np,
        }
        if with_bias:
            bq = ba[lo:hi] * scale
            bk = ba[C + lo:C + hi]
            bv = ba[2 * C + lo:2 * C + hi]
            m["bqkv"] = np.concatenate([bq, bk, bv])[None, :].astype(NPBF16)
            m["ones512"] = ones512_np
        in_maps.append(m)
    return in_maps, with_bias


def _combine(results, b_proj):
    acc = np.zeros((B, T, C), dtype=np.float32)
    for r in results:
        acc += np.asarray(r["out"], dtype=np.float32)
    acc += np.asarray(b_proj, dtype=np.float32)[None, None, :]
    return np.ascontiguousarray(acc)


def run(x, w_attn, b_attn, w_proj, b_proj, trace=False, trace_cores=None):
    in_maps, with_bias = _prep_inputs(x, w_attn, b_attn, w_proj)
    nc = _get_nc(with_bias)
    res = run_bass_kernel_spmd(
        nc, in_maps, core_ids=list(range(NCORES)),
        trace=trace, trace_cores=trace_cores,
    )
    return _combine(res.results, b_proj), res


def kernel(x, w_attn, b_attn, w_proj, b_proj):
    out, _ = run(x, w_attn, b_attn, w_proj, b_proj, trace=False)
    return out


# revision 52
# speedup vs baseline: 1.0183x; 1.0037x over previous
"""Causal self-attention Trainium2 kernel (8-core head-parallel).

Full inputs in, full output out. Sharding strategy:
  - 16 heads / 8 cores -> 2 heads per core, both batch elems (4 (b,h) attention
    problems per core).
  - QKV projection column-parallel: each core gets w_attn[:, cols-of-its-heads]
    as a [1024, 384] slice (q 128 | k 128 | v 128), q pre-scaled by 1/sqrt(D).
  - c_proj row-parallel: each core gets w_proj[128c:128c+128, :] and produces a
    partial [B, T, C] output; host sums the 8 partials (the all-reduce of the
    row-parallel projection) and adds the bias.

Schedule (single phase-A + phase-B pipeline, no fat warm-up matmuls):
  Phase A: x^T streams in token-major [128, 512/1024] pieces; QKV runs in
  512-token groups with q/k/v matmuls interleaved per contraction tile so the
  PE is fed as DMA lands.  v^T tiles are PE-transposed into a V slab
  [128 s, 32, 130] (h0 d | ones | h1 d) so each AV stationary is a contiguous
  65-column slice whose ones column yields the softmax denominator Z for free.
  Phase B: attention in 512-wide q windows; both heads share one [128,2,512]
  S psum tile -> one exp per s-tile; causal masking via a triangular multiply
  on GPSIMD for diagonal tiles only.  Normalization uses DVE reciprocal on the
  Z row + a PE broadcast matmul.  Output projection takes normalized y^T as
  stationary and w_proj rows as moving, producing out[b, tok, :] tiles
  directly in [B, T, C] orientation.
"""

import math

import numpy as np
import ml_dtypes

import concourse.bass as bass
from concourse import bacc
import concourse.mybir as mybir
from concourse.tile import TileContext
from concourse.bass_utils import run_bass_kernel_spmd

BF16 = mybir.dt.bfloat16
F32 = mybir.dt.float32
NPBF16 = ml_dtypes.bfloat16

P = 128
B, T, C = 2, 2048, 1024
H, D = 16, 64
NCORES = 8
HPC = H // NCORES          # heads per core
TOK = B * T                # 4096 flattened tokens (b-major)
NCT = C // P               # 8 contraction tiles for the projections
QW = 512                   # q window width for attention
NG = TOK // QW             # 8 token groups of 512 for the QKV projection
NST = TOK // P             # 32 token tiles of 128
EXP_BIAS = -4.0            # exp(s - 4): cancels in normalization, guards tail


def build_nc(with_bias: bool) -> bacc.Bacc:
    nc = bacc.Bacc(None, target_bir_lowering=False)

    xt = nc.dram_tensor("xt", [C, TOK], BF16, kind="ExternalInput")
    wqkv = nc.dram_tensor("wqkv", [C, 3 * P], BF16, kind="ExternalInput")
    wp = nc.dram_tensor("wp", [P, C], BF16, kind="ExternalInput")
    tri = nc.dram_tensor("tri", [P, P], BF16, kind="ExternalInput")
    ident = nc.dram_tensor("ident", [P, P], BF16, kind="ExternalInput")
    ones64 = nc.dram_tensor("ones64", [1, 64], BF16, kind="ExternalInput")
    if with_bias:
        bqkv = nc.dram_tensor("bqkv", [1, 3 * P], BF16, kind="ExternalInput")
        ones512 = nc.dram_tensor("ones512", [1, 512], BF16, kind="ExternalInput")
    out = nc.dram_tensor("out", [B, T, C], BF16, kind="ExternalOutput")

    EXP = mybir.ActivationFunctionType.Exp

    with TileContext(nc) as tc:
        with (
            tc.tile_pool(name="consts", bufs=1) as consts,
            tc.tile_pool(name="px", bufs=1) as px,
            tc.tile_pool(name="pqkv", bufs=1) as pqkv,
            tc.tile_pool(name="py", bufs=1) as py,
            tc.tile_pool(name="pwork", bufs=2) as pwork,
        ):
            # ---- constant loads on the ACT queue (xt owns the sync queue) ----
            # one fused DMA: dram row ct*P+p -> sbuf partition p, free (ct, :)
            wqkv_sb = consts.tile([P, NCT, 3 * P], BF16)
            nc.scalar.dma_start(
                wqkv_sb, wqkv[:, :].rearrange("(ct p) f -> p ct f", ct=NCT))
            wp_sb = consts.tile([P, C], BF16)
            nc.scalar.dma_start(wp_sb, wp[:, :])
            tri_sb = consts.tile([P, P], BF16)
            nc.scalar.dma_start(tri_sb, tri[:, :])
            ident_sb = consts.tile([P, P], BF16)
            nc.scalar.dma_start(ident_sb, ident[:, :])
            ones64_sb = consts.tile([1, 64], BF16)
            nc.scalar.dma_start(ones64_sb, ones64[:, :])
            expb = consts.tile([P, 1], F32)
            nc.vector.memset(expb, EXP_BIAS)
            warm_sb = consts.tile([P, P], BF16)
            nc.vector.memset(warm_sb, 1.0)
            if with_bias:
                bqkv_sb = consts.tile([1, 3 * P], BF16)
                nc.scalar.dma_start(bqkv_sb, bqkv[:, :])
                ones512_sb = consts.tile([1, 512], BF16)
                nc.scalar.dma_start(ones512_sb, ones512[:, :])

            # ---- x^T streamed token-major over two issue queues (sync takes
            # even ct, ACT odd ct) so descriptor-issue rate never starves the
            # QKV groups and transfers spread across DMA engines ----
            xt_sb = px.tile([P, NCT, TOK], BF16)
            for lo, hi in ((0, 512), (512, 1024), (1024, 2048),
                           (2048, 4096)):
                for ct in range(NCT):
                    eng = nc.sync if ct % 2 == 0 else nc.scalar
                    eng.dma_start(xt_sb[:, ct, lo:hi],
                                  xt[ct * P:(ct + 1) * P, lo:hi])

            # qkv^T slab: ft 0 q^T, 1 k^T, 2 v^T
            qkT = pqkv.tile([P, 3, TOK], BF16)
            vT = qkT[:, 2, :]
            # V slab: [s, tile, 131] = h0 d 0:64 | ones | h1 d 65:129 | ones
            V = pqkv.tile([P, NST, 131], BF16)
            nc.vector.memset(V[:, :, 64:65], 1.0)
            nc.vector.memset(V[:, :, 129:130], 1.0)
            yT = py.tile([P, B, T], BF16)

            # ---- phase A: QKV projection, token-major groups ----
            with (
                tc.tile_pool(name="ps_qkv", bufs=1, space="PSUM") as ps_qkv,
                tc.tile_pool(name="ps_t", bufs=1, space="PSUM") as ps_t,
            ):
                wn = [0]

                def warm(n):
                    # PE warm-up that draws little power: transpose matmuls of
                    # an sbuf constant (pass-through, no MAC toggling). The
                    # tiny DVE read defeats walrus dead-code elimination.
                    for _ in range(n):
                        wn[0] += 1
                        pt = ps_t.tile([P, P], BF16, tag="vt", bufs=2,
                                       name=f"warmps_{wn[0]}")
                        nc.tensor.transpose(pt, warm_sb, warm_sb)
                        wr = pwork.tile([1, 4], BF16, tag="wr", bufs=2,
                                        name=f"wr_{wn[0]}")
                        nc.vector.tensor_copy(wr, pt[0:1, 0:4])

                warm(12)

                for g in range(NG):
                    gl, gh = g * QW, (g + 1) * QW
                    ps3 = ps_qkv.tile([P, 3, QW], F32, tag="qkv", bufs=2,
                                      name=f"qkvps_{g}")
                    for ct in range(NCT):
                        for ft in range(3):
                            nc.tensor.matmul(
                                ps3[:, ft, :],
                                wqkv_sb[:, ct, ft * P:(ft + 1) * P],
                                xt_sb[:, ct, gl:gh],
                                start=(ct == 0),
                                stop=(ct == NCT - 1 and not with_bias),
                            )
                        if g <= 1:
                            warm(1)
                    if with_bias:
                        for ft in range(3):
                            nc.tensor.matmul(
                                ps3[:, ft, :],
                                bqkv_sb[0:1, ft * P:(ft + 1) * P],
                                ones512_sb[0:1, :],
                                start=False, stop=True,
                            )
                    nc.scalar.copy(qkT[:, :, gl:gh], ps3)
                    # v^T -> V slab tiles for this group (both heads per tile)
                    for st4 in range(QW // P):
                        si = g * (QW // P) + st4
                        pt = ps_t.tile([P, P], BF16, tag="vt", bufs=2,
                                       name=f"vtps_{si}")
                        nc.tensor.transpose(
                            pt, vT[:, si * P:(si + 1) * P], ident_sb)
                        nc.vector.tensor_copy(V[:, si, 0:64], pt[:, 0:64])
                        nc.vector.tensor_copy(V[:, si, 65:129], pt[:, 64:128])

            # ---- phase B: attention + normalize + projection ----
            with tc.tile_pool(name="ps_att", bufs=1, space="PSUM") as ps_att:
                pending = []           # deferred norm+proj stages of prev window
                otn = [0]

                def emit_proj(b, w, tts=None):
                    qbase = w * QW
                    for tt in (range(QW // P) if tts is None else tts):
                        t0 = qbase + tt * P
                        pp = ps_att.tile([P, QW], F32, tag="proj", bufs=2,
                                         name=f"pp_{b}_{w}_{tt}")
                        ot = pwork.tile([P, C], BF16, tag="ot", bufs=6,
                                        name=f"ot_{b}_{w}_{tt}")
                        for c in range(2):
                            nc.tensor.matmul(
                                pp[:, 0:512],
                                yT[:, b, t0:t0 + P],
                                wp_sb[:, c * 512:(c + 1) * 512],
                                start=True, stop=True,
                            )
                            if otn[0] % 3 != 2:
                                nc.vector.tensor_copy(
                                    ot[:, c * 512:(c + 1) * 512], pp[:, 0:512])
                            else:
                                nc.scalar.copy(
                                    ot[:, c * 512:(c + 1) * 512], pp[:, 0:512])
                            otn[0] += 1
                        nc.sync.dma_start(out[b, t0:t0 + P, :], ot)

                def make_norm(b, w, ys):
                    """Stage closures: [recip+rb, ynum+mul+proj]."""
                    qbase = w * QW
                    rs, rbs = [], []

                    def stage1():
                        for h in range(HPC):
                            # approx-recip ucode breaks on non-zero partition
                            # base: stage Z at partition 0 first
                            zr = pwork.tile([1, QW], F32, tag="zr", bufs=4,
                                            name=f"zr_{b}_{w}_{h}")
                            nc.vector.tensor_copy(zr, ys[h][64:65, :])
                            rf = pwork.tile([1, QW], F32, tag="rf", bufs=4,
                                            name=f"rf_{b}_{w}_{h}")
                            nc.vector.reciprocal_approx_fast(rf, zr)
                            r = pwork.tile([1, QW], BF16, tag="r", bufs=4,
                                           name=f"r_{b}_{w}_{h}")
                            nc.scalar.copy(r, rf)
                            rb = ps_att.tile([P, QW], F32, tag="proj",
                                             bufs=2, name=f"rb_{b}_{w}_{h}")
                            nc.tensor.matmul(rb[0:64, :], ones64_sb, r,
                                             start=True, stop=True)
                            rs.append(r)
                            rbs.append(rb)

                    def stage2(c0=0, c1=QW, tts=None):
                        for h in range(HPC):
                            ynum = pwork.tile([64, c1 - c0], BF16, tag="ynum",
                                              bufs=4,
                                              name=f"yn_{b}_{w}_{h}_{c0}")
                            nc.vector.tensor_copy(ynum, ys[h][0:64, c0:c1])
                            nc.vector.tensor_mul(
                                yT[h * 64:(h + 1) * 64, b,
                                   qbase + c0:qbase + c1],
                                ynum, rbs[h][0:64, c0:c1])
                        emit_proj(b, w, tts)

                    if (b, w) == (B - 1, T // QW - 1):
                        # final window: drain in halves so the first output
                        # projections and DMAs overlap the second normalize
                        return [stage1,
                                lambda: stage2(0, QW // 2, (0, 1)),
                                lambda: stage2(QW // 2, QW, (2, 3))]
                    return [stage1, stage2]

                for b in range(B):
                    for w in range(T // QW):
                        qbase = w * QW
                        n_st = (qbase + QW) // P
                        # lazy y allocation: first AV (emitted after the
                        # previous window's norm stages) grabs the slots, so
                        # slot reuse correctly waits on those stages' reads
                        ys = {}

                        def get_ys(h, b=b, w=w, ys=ys):
                            if h not in ys:
                                ys[h] = ps_att.tile(
                                    [P, QW], F32, tag="y", bufs=2,
                                    name=f"yps_{b}_{w}_{h}")
                            return ys[h]

                        av_fifo = []
                        for st in range(n_st):
                            if pending and st in (0, 2):
                                pending.pop(0)()
                            s0 = st * P
                            qa = max(qbase, s0)
                            wdt = qbase + QW - qa
                            ps = ps_att.tile([P, 2, QW], F32, tag="S", bufs=2,
                                             name=f"sps_{b}_{w}_{st}")
                            for h in range(HPC):
                                nc.tensor.matmul(
                                    ps[:, h, 0:wdt],
                                    qkT[h * 64:(h + 1) * 64, 1,
                                        b * T + s0: b * T + s0 + P],
                                    qkT[h * 64:(h + 1) * 64, 0,
                                        b * T + qa: b * T + qa + wdt],
                                    start=True, stop=True,
                                )
                            es = pwork.tile([P, 2, QW], BF16, tag="es", bufs=8,
                                            name=f"es_{b}_{w}_{st}")
                            nc.scalar.activation(
                                es[:, :, 0:wdt], ps[:, :, 0:wdt], EXP,
                                bias=expb)
                            if s0 >= qbase:      # diagonal tile: causal mask
                                # DVE, all-bf16 SBUF operands -> 4x fast path
                                for h in range(HPC):
                                    nc.vector.tensor_mul(
                                        es[:, h, 0:P], es[:, h, 0:P], tri_sb)

                            def av(st=st, es=es, off=qa - qbase, wdt=wdt,
                                   last=(st == n_st - 1), first=(st == 0),
                                   vi=b * (T // P) + st, get_ys=get_ys):
                                for h in range(HPC):
                                    nc.tensor.matmul(
                                        get_ys(h)[0:65, off:off + wdt],
                                        V[:, vi, h * 65: h * 65 + 65],
                                        es[:, h, 0:wdt],
                                        start=first, stop=last,
                                    )
                            av_fifo.append(av)
                            if len(av_fifo) > 3:
                                av_fifo.pop(0)()
                        for f in av_fifo:
                            f()
                        pending = make_norm(b, w, [ys[h] for h in range(HPC)])
                for f in pending:
                    f()
    nc.compile()
    return nc


_CACHE = {}


def _get_nc(with_bias: bool) -> bacc.Bacc:
    if with_bias not in _CACHE:
        _CACHE[with_bias] = build_nc(with_bias)
    return _CACHE[with_bias]


def _prep_inputs(x, w_attn, b_attn, w_proj):
    """Host-side shard + layout prep. Returns per-core in_maps."""
    xf = np.ascontiguousarray(
        np.asarray(x, dtype=np.float32).reshape(TOK, C).T
    ).astype(NPBF16)                                   # x^T [C, TOK]
    w = np.asarray(w_attn, dtype=np.float32)
    ba = np.asarray(b_attn, dtype=np.float32)
    wpj = np.asarray(w_proj, dtype=np.float32)
    scale = 1.0 / math.sqrt(D)
    with_bias = bool(np.any(ba))

    tri_np = np.triu(np.ones((P, P), dtype=np.float32)).astype(NPBF16)
    id_np = np.eye(P, dtype=np.float32).astype(NPBF16)
    ones64_np = np.ones((1, 64), dtype=np.float32).astype(NPBF16)
    ones512_np = np.ones((1, 512), dtype=np.float32).astype(NPBF16)

    in_maps = []
    for c in range(NCORES):
        lo, hi = c * HPC * D, (c + 1) * HPC * D        # 128-wide head slice
        wq = w[:, lo:hi] * scale
        wk = w[:, C + lo:C + hi]
        wv = w[:, 2 * C + lo:2 * C + hi]
        wqkv_c = np.concatenate([wq, wk, wv], axis=1).astype(NPBF16)
        wp_c = np.ascontiguousarray(wpj[lo:hi, :]).astype(NPBF16)
        m = {
            "xt": xf,
            "wqkv": wqkv_c,
            "wp": wp_c,
            "tri": tri_np,
            "ident": id_np,
            "ones64": ones64_np,
        }
        if with_bias:
            bq = ba[lo:hi] * scale
            bk = ba[C + lo:C + hi]
            bv = ba[2 * C + lo:2 * C + hi]
            m["bqkv"] = np.concatenate([bq, bk, bv])[None, :].astype(NPBF16)
            m["ones512"] = ones512_np
        in_maps.append(m)
    return in_maps, with_bias


def _combine(results, b_proj):
    acc = np.zeros((B, T, C), dtype=np.float32)
    for r in results:
        acc += np.asarray(r["out"], dtype=np.float32)
    acc += np.asarray(b_proj, dtype=np.float32)[None, None, :]
    return np.ascontiguousarray(acc)


def run(x, w_attn, b_attn, w_proj, b_proj, trace=False, trace_cores=None):
    in_maps, with_bias = _prep_inputs(x, w_attn, b_attn, w_proj)
    nc = _get_nc(with_bias)
    res = run_bass_kernel_spmd(
        nc, in_maps, core_ids=list(range(NCORES)),
        trace=trace, trace_cores=trace_cores,
    )
    return _combine(res.results, b_proj), res


def kernel(x, w_attn, b_attn, w_proj, b_proj):
    out, _ = run(x, w_attn, b_attn, w_proj, b_proj, trace=False)
    return out
